# revision 54
# baseline (speedup 1.0000x reference)
"""Trainium2 Bass kernel for nn_Attention_43868795961547 (sparse_attention).

Reference computation per batch item (8 items, data-parallel over 8 cores):
  x  = LN(img[b]) @ w_qkv -> q,k,v (8 heads x 64)          [1024 tokens]
  kt,vt from LN(tab[b]) @ w_tab_qkv appended as key/value position 1024
  out = softmax(q k^T / 8) @ v ; out @ w_out + b_out        -> [1024, 512]

Strategy (per core), ~192us vs the 214us f32r baseline:
  - all matmul operands bf16 (same 1 cyc/row PE rate as f32r at N>=512;
    rel err 7.5e-3 vs the 2e-2 gate), PSUM fp32.
  - LN fully in token-major form (fused scalar_tensor_tensor pairs), xnT
    via merged XBAR dma_start_transpose calls (one per token tile).
    IMPORTANT: dma_start_transpose completion is NOT dependency-tracked by
    the tile framework; consumers race it.  The xnT chunks are therefore
    landed through ordinary (tracked) SBUF->SBUF DMAs on the same queue.
  - tab dots via a zero-padded block-diagonal lhsT (bf16 matmuls with
    col tile_position offsets return garbage, so no tile_position games).
  - tab k/v contribution to attn@v folded into the PSUM eviction with DVE
    scalar_tensor_tensor (no K=1 PE matmuls); tab exp rows broadcast via
    DRAM bounce (gpsimd partition_broadcast ignores nonzero input
    partition offsets on hardware).
  - denominators ride along as an interleaved ones column in v (M=65
    attn@v output row 64), reciprocal + DRAM-bounce broadcast, normalize.
  - attention inner loop interleaves dots (ACT-coupled via exp) with
    attn@v accumulation (PE-only) so the PE never outruns the exp stream:
    per head pair d0 d1 a0 d2 a1 d3 a2 a3.
  - engine budget: PE ~107us matmuls; ACT ~68us exp (the per-core floor:
    8.4M exps at ~1 elem/cycle/partition) + weight casts; DVE evictions/
    LN/normalize; GpSimd idle-ish (no PSUM access, no TensorScalarPtr).
"""

import numpy as np

import concourse.bass as bass
import concourse.mybir as mybir
import concourse.tile as tile
from concourse import bacc
from concourse import bass_utils
from concourse.masks import make_identity

F32 = mybir.dt.float32
BF16 = mybir.dt.bfloat16

N_CORES = 8
NTOK = 1024  # img tokens per batch item
DIM = 512
HEADS = 8
DHEAD = 64
INNER = 512
SCALE = DHEAD ** -0.5  # 0.125
EPS = 1e-5

NT = NTOK // 128   # 8 token tiles
NC_ = DIM // 128   # 4 feature chunks
NQB = 2            # q blocks of 512
QB = 512

import os
PHASE_LIMIT = int(os.environ.get("KERNEL_PHASE_LIMIT", "99"))


def build_program():
    nc = bacc.Bacc(
        "TRN2",
        target_bir_lowering=False,
        debug=False,
        enable_asserts=False,
        num_devices=N_CORES,
    )

    img = nc.dram_tensor("img_s", [NTOK, DIM], F32, kind="ExternalInput").ap()
    tab = nc.dram_tensor("tab_s", [1, DIM], F32, kind="ExternalInput").ap()
    w_qkv = nc.dram_tensor("w_qkv", [DIM, 3 * INNER], F32, kind="ExternalInput").ap()
    # only k,v columns of w_tab_qkv (cols 512:1536) are used
    w_tab = nc.dram_tensor("w_tab", [DIM, 2 * INNER], F32, kind="ExternalInput").ap()
    w_out = nc.dram_tensor("w_out", [INNER, DIM], F32, kind="ExternalInput").ap()
    b_out = nc.dram_tensor("b_out", [1, DIM], F32, kind="ExternalInput").ap()
    ln_w = nc.dram_tensor("ln_w", [1, DIM], F32, kind="ExternalInput").ap()
    ln_b = nc.dram_tensor("ln_b", [1, DIM], F32, kind="ExternalInput").ap()
    out_d = nc.dram_tensor("out_s", [NTOK, DIM], F32, kind="ExternalOutput").ap()
    tdscr = nc.dram_tensor("tdscr", [HEADS, NTOK], BF16, kind="Internal").ap()
    rdscr = nc.dram_tensor("rdscr", [HEADS, NTOK], F32, kind="Internal").ap()

    with tile.TileContext(nc) as tc:
        kernel_body(tc, img, tab, w_qkv, w_tab, w_out, b_out, ln_w, ln_b, out_d,
                    tdscr, rdscr)

    nc.compile()
    return nc


def kernel_body(tc, img, tab, w_qkv, w_tab, w_out, b_out, ln_w, ln_b, out_d,
                tdscr, rdscr):
    nc = tc.nc
    AF = mybir.ActivationFunctionType
    OP = mybir.AluOpType

    import contextlib
    ctx = contextlib.ExitStack()
    with ctx:
        # ---------------- persistent pools ----------------
        const_p = ctx.enter_context(tc.tile_pool(name="const", bufs=1))
        qkT_p = ctx.enter_context(tc.tile_pool(name="qkT", bufs=1))
        v_p = ctx.enter_context(tc.tile_pool(name="vp", bufs=1))
        outuT_p = ctx.enter_context(tc.tile_pool(name="outuT", bufs=1))
        small_p = ctx.enter_context(tc.tile_pool(name="smallp", bufs=1))
        wo_p = ctx.enter_context(tc.tile_pool(name="wop", bufs=1))

        # psum pools (8 banks total):
        #  big: 3 tags x [128,1024] (2 banks each) = 6 banks
        #  o:   [65,512] 1 bank  (attn@v out)
        #  fin: [128,512] 1 bank (final matmul / micro transposes)
        psum_big = ctx.enter_context(tc.tile_pool(name="psbig", bufs=1, space="PSUM"))
        psum_o = ctx.enter_context(tc.tile_pool(name="pso", bufs=1, space="PSUM"))
        psum_f = ctx.enter_context(tc.tile_pool(name="psf", bufs=1, space="PSUM"))
        bigctr = [0]

        def big_tile(name):
            t = psum_big.tile([128, 2 * QB], F32, name=name,
                              tag=f"big{bigctr[0] % 2}")
            bigctr[0] += 1
            return t

        # ---------------- constants ----------------
        ident = const_p.tile([128, 128], F32, name="ident")
        make_identity(nc, ident)

        eps_t = const_p.tile([128, 1], F32, name="eps_t")
        nc.vector.memset(eps_t, EPS)

        lnw_bc = const_p.tile([128, DIM], F32, name="lnw_bc")
        lnb_bc = const_p.tile([128, DIM], F32, name="lnb_bc")
        bout_bc = const_p.tile([128, DIM], F32, name="bout_bc")
        for dst, src in ((lnw_bc, ln_w), (lnb_bc, ln_b), (bout_bc, b_out)):
            nc.gpsimd.dma_start(out=dst, in_=src.to_broadcast([128, DIM]))

        ones8 = const_p.tile([128, 8], BF16, name="ones8")
        nc.vector.memset(ones8, 1.0)

        # w_out chunks, persistent bf16
        wo = [wo_p.tile([128, DIM], BF16, name=f"wo{c}", tag=f"wo{c}")
              for c in range(NC_)]

        # ---------------- persistent activations ----------------
        qT = [qkT_p.tile([128, NTOK], BF16, name=f"qT{c}", tag=f"qT{c}") for c in range(NC_)]
        kT = [qkT_p.tile([128, NTOK], BF16, name=f"kT{c}", tag=f"kT{c}") for c in range(NC_)]
        # v token-major with interleaved ones column per head: 8 x (64+1) = 520
        v_sb = [v_p.tile([128, 520], BF16, name=f"v{t}", tag=f"v{t}") for t in range(NT)]
        # unnormalized out^T chunks [128, 1024] (bf16; final matmul lhsT)
        outuT = [outuT_p.tile([128, NTOK], BF16, name=f"ouT{c}", tag=f"ouT{c}") for c in range(NC_)]

        # feature-major tab projections: cols 0..3 = k_t chunks, 4..7 = v_t
        kv_tT = small_p.tile([128, 8], BF16, name="kv_tT")
        # denominator collectors: rows 32*j of dcol[g] = head 4g+j
        dcol = [small_p.tile([128, NTOK], F32, name=f"dcol{g}") for g in range(2)]
        rcol = [small_p.tile([128, NTOK], F32, name=f"rcol{g}") for g in range(2)]
        rscratch = small_p.tile([128, NTOK], F32, name="rscratch")
        for g in range(2):
            nc.vector.memset(dcol[g], 1.0)
        # tab-dot exp rows: tabexp[g] rows 32j = head 4g+j, all 1024 q cols
        tabexp = [small_p.tile([128, NTOK], BF16, name=f"tabexp{g}") for g in range(2)]
        # tab exp broadcast per head pair: bctab[hp] rows 0:64 = head 2hp,
        # rows 64:128 = head 2hp+1 (all q cols)
        bctab = [small_p.tile([128, NTOK], BF16, name=f"bctab{hp}") for hp in range(4)]

        # ---------------- phase 1+2: LN, transpose, qkv (scoped weights) ----
        with tc.tile_pool(name="wqkv", bufs=1) as wq_p, \
             tc.tile_pool(name="xnt", bufs=1) as xnT_p, \
             tc.tile_pool(name="lnt", bufs=2) as ln_p:

            # img tiles first: LN starts immediately; weights stream behind
            x_ts = []
            tb = ln_p.tile([1, DIM], F32, name="tb", tag="tb", bufs=1)
            nc.gpsimd.dma_start(out=tb, in_=tab)
            for t in range(NT):
                x_t = ln_p.tile([128, DIM], F32, name="x_t", tag="x_t", bufs=8)
                eng = nc.sync if t < 4 else nc.scalar
                eng.dma_start(out=x_t, in_=img[t * 128:(t + 1) * 128, :])
                x_ts.append(x_t)

            wq_st, wt_st, wo_st = [], [], []
            for c in range(NC_):
                t = wq_p.tile([128, 3 * INNER], F32, name=f"wqs{c}", tag=f"wqs{c}")
                nc.scalar.dma_start(out=t, in_=w_qkv[c * 128:(c + 1) * 128, :])
                wq_st.append(t)

            # (t, c)-major transpose target so PAIRS of token tiles can be
            # transposed in one XBAR call (the call cost is ~1.2us fixed)
            xnT_all = xnT_p.tile([128, NT, NC_, 128], BF16, name="xnT_all",
                                 tag="xnT_all")
            xn_all = xnT_p.tile([128, NT, DIM], BF16, name="xn_all",
                                tag="xn_all")
            xnT = [xnT_p.tile([128, NTOK], BF16, name=f"xnT{c}", tag=f"xnT{c}")
                   for c in range(NC_)]

            # --- fused per-tile LN: stats -> sqrt -> normalize -> transpose
            # (per-tile emission self-pipelines across DVE/ACT; the old
            # pass-split serialized tile 0's normalize behind tile 7's stats)
            for t in range(NT):
                stats = ln_p.tile([128, 6], F32, name="stats", tag="stats")
                nc.vector.bn_stats(out=stats, in_=x_ts[t])
                mv = ln_p.tile([128, 2], F32, name="mv", tag="mv", bufs=8)
                nc.vector.bn_aggr(out=mv, in_=stats)
                sd = ln_p.tile([128, 1], F32, name="sd", tag="sd", bufs=8)
                nc.scalar.activation(out=sd, in_=mv[:, 1:2], func=AF.Sqrt,
                                     bias=eps_t, scale=1.0)
                rstd = ln_p.tile([128, 1], F32, name="rstd", tag="rstd")
                nc.vector.reciprocal(out=rstd, in_=sd)
                tmp_t = ln_p.tile([128, DIM], F32, name="tmp_t", tag=f"tmp{t % 2}")
                nc.vector.scalar_tensor_tensor(
                    out=tmp_t, in0=x_ts[t], scalar=mv[:, 0:1], in1=lnw_bc,
                    op0=OP.subtract, op1=OP.mult)
                nc.vector.scalar_tensor_tensor(
                    out=xn_all[:, t, :], in0=tmp_t, scalar=rstd, in1=lnb_bc,
                    op0=OP.mult, op1=OP.add)
                if t % 2 == 1:
                    nc.sync.dma_start_transpose(
                        out=xnT_all[:, t - 1:t + 1, :, :],
                        in_=xn_all[:, t - 1:t + 1, :])

            # land xnT chunks as separate tiles via ordinary (tracked) DMAs
            # on the same queue as the (untracked) XBAR transposes
            for c in range(NC_):
                nc.sync.dma_start(out=xnT[c], in_=xnT_all[:, :, c, :])

            # bf16 wq cast on ACT (idle early; ordered after the wq loads
            # on the same queue)
            wq = []
            for c in range(NC_):
                t = wq_p.tile([128, 3 * INNER], BF16, name=f"wq{c}", tag=f"wq{c}")
                nc.scalar.copy(out=t, in_=wq_st[c])
                wq.append(t)
            wq_keep = wq[0]

            if PHASE_LIMIT <= 1:
                with tc.tile_pool(name="dbg", bufs=2) as dbg_p:
                    for t in range(NT):
                        fo = dbg_p.tile([128, DIM], F32, name="dbg_fo", tag="fo")
                        nc.vector.tensor_copy(out=fo, in_=xnT[t % 4][:, 0:DIM])
                        nc.sync.dma_start(out=out_d[t * 128:(t + 1) * 128, :], in_=fo)
                return

            for c in range(NC_):
                t = wq_p.tile([128, 2 * INNER], mybir.dt.float32r,
                              name=f"wts{c}", tag=f"wts{c}")
                nc.vector.tensor_copy(out=t[0:1, 0:1],
                                      in_=eps_t[0:1, 0:1].bitcast(mybir.dt.float32r))
                nc.sync.dma_start(out=t,
                                  in_=w_tab[c * 128:(c + 1) * 128, :].bitcast(mybir.dt.float32r))
                wt_st.append(t)
            for c in range(NC_):
                t = wq_p.tile([128, DIM], F32, name=f"wos{c}", tag=f"wos{c}")
                nc.vector.memset(t[0:1, 0:1], 0.0)
                nc.sync.dma_start(out=t, in_=w_out[c * 128:(c + 1) * 128, :])
                wo_st.append(t)
            for c in range(NC_):
                nc.gpsimd.tensor_copy(out=wo[c], in_=wo_st[c])

            # --- qT, kT: feature-major qkv ---
            # order: head-pair 0's q+k first so dots can start early
            for m in (0, 4, 1, 5, 2, 6, 3, 7):
                dst = qT[m] if m < 4 else kT[m - 4]
                for qb in range(NQB):
                    ps = big_tile("psqk")
                    half = ps[:, 0:QB] if qb == 0 else ps[:, QB:2 * QB]
                    for kc in range(NC_):
                        nc.tensor.matmul(
                            half,
                            lhsT=wq[kc][:, m * 128:(m + 1) * 128],
                            rhs=xnT[kc][:, qb * QB:(qb + 1) * QB],
                            start=(kc == 0), stop=(kc == NC_ - 1))
                    nc.vector.tensor_copy(out=dst[:, qb * QB:(qb + 1) * QB], in_=half)

            # --- v token-major (+ ones interleave) ---
            for t in range(NT):
                ps = big_tile("psv")
                pv = ps[:, 0:QB]
                for kc in range(NC_):
                    nc.tensor.matmul(
                        pv,
                        lhsT=xnT[kc][:, t * 128:(t + 1) * 128],
                        rhs=wq[kc][:, 2 * INNER:3 * INNER],
                        start=(kc == 0), stop=(kc == NC_ - 1))
                vdst = v_sb[t].rearrange("p (h s) -> p h s", s=65)
                nc.vector.tensor_copy(out=vdst[:, :, 0:64],
                                      in_=pv.rearrange("p (h d) -> p h d", d=64))
                nc.vector.tensor_copy(
                    out=vdst[:, :, 64:65],
                    in_=ones8.rearrange("p (h o) -> p h o", o=1))

            # --- tab layernorm (1 row; tb loaded early) ---
            tstats = ln_p.tile([1, 6], F32, name="tstats", tag="tstats")
            nc.vector.bn_stats(out=tstats, in_=tb)
            tmv = ln_p.tile([1, 2], F32, name="tmv", tag="tmv")
            nc.vector.bn_aggr(out=tmv, in_=tstats)
            tsd = ln_p.tile([1, 1], F32, name="tsd", tag="tsd")
            nc.scalar.activation(out=tsd, in_=tmv[:, 1:2], func=AF.Sqrt,
                                 bias=eps_t[0:1], scale=1.0)
            trstd = ln_p.tile([1, 1], F32, name="trstd", tag="trstd")
            nc.vector.reciprocal(out=trstd, in_=tsd)
            ttmp = ln_p.tile([1, DIM], F32, name="ttmp", tag="ttmp", bufs=1)
            nc.vector.scalar_tensor_tensor(
                out=ttmp, in0=tb, scalar=tmv[:, 0:1], in1=lnw_bc[0:1, :],
                op0=OP.subtract, op1=OP.mult)
            tn = ln_p.tile([1, DIM], F32, name="tn", tag="tn", bufs=1)
            nc.vector.scalar_tensor_tensor(
                out=tn, in0=ttmp, scalar=trstd, in1=lnb_bc[0:1, :],
                op0=OP.mult, op1=OP.add)

            # transpose tn -> tnT [512,1] as 4 chunks [128,1] (PE micro)
            tnT = ln_p.tile([128, NC_], mybir.dt.float32r, name="tnT", tag="tnT", bufs=1)
            for c in range(NC_):
                pt = psum_f.tile([128, 1], F32, name="ptn", tag="fin")
                nc.tensor.transpose(out=pt, in_=tn[0:1, c * 128:(c + 1) * 128],
                                    identity=ident[0:1, 0:1])
                nc.vector.tensor_copy(out=tnT[:, c:c + 1], in_=pt)

            # --- tab k/v: one row matmul [1, 1024] (k_t | v_t) ---
            ps_kv = big_tile("pskv")
            kv_row_ps = ps_kv[0:1, :]
            for half in range(2):
                for kc in range(NC_):
                    nc.tensor.matmul(
                        kv_row_ps[:, half * QB:(half + 1) * QB],
                        lhsT=tnT[:, kc:kc + 1],
                        rhs=wt_st[kc][:, half * QB:(half + 1) * QB],
                        start=(kc == 0), stop=(kc == NC_ - 1))
            kv_row = ln_p.tile([1, 2 * INNER], F32, name="kv_row", tag="kvr", bufs=1)
            nc.vector.tensor_copy(out=kv_row, in_=kv_row_ps)
            # feature-major: kv_tT col c (c<4: k_t chunk c; c>=4: v_t chunk c-4)
            for c in range(8):
                pt = psum_f.tile([128, 1], F32, name="pkvt", tag="fin")
                nc.tensor.transpose(out=pt, in_=kv_row[0:1, c * 128:(c + 1) * 128],
                                    identity=ident[0:1, 0:1])
                nc.vector.tensor_copy(out=kv_tT[:, c:c + 1], in_=pt)

        if PHASE_LIMIT <= 2:
            wq_dbg = wq_keep
            with tc.tile_pool(name="dbg", bufs=2) as dbg_p:
                for t in range(NT):
                    fo = dbg_p.tile([128, DIM], F32, name="dbg_fo", tag="fo")
                    if t < 4:
                        nc.vector.tensor_copy(out=fo, in_=xnT[t][:, 0:DIM])
                    else:
                        nc.vector.tensor_copy(out=fo, in_=qT[t % 4][:, 0:DIM])
                    nc.sync.dma_start(out=out_d[t * 128:(t + 1) * 128, :], in_=fo)
            return

        # ---------------- phase 3: tab dots + exp + broadcasts ----------------
        # Block-diagonal lhsT per (g, c): head h = 4g+j has k_t feats in
        # chunk c = h//2 at rows (h%2)*64, placed at column 32*j so the
        # matmul output lands on row 32*j of tabd[g]. No tile_position games.
        bd = small_p.tile([128, 4, 128], BF16, name="bd_tab")
        for c in range(NC_):
            nc.vector.memset(bd[:, c, :], 0.0)
        for g in range(2):
            for j in range(4):
                h = 4 * g + j
                c, hb = h // 2, (h % 2) * 64
                nc.vector.tensor_copy(
                    out=bd[hb:hb + 64, c, 32 * j:32 * j + 1],
                    in_=kv_tT[hb:hb + 64, c:c + 1])
        for g in range(2):
            ps = big_tile("pstd")
            for half in range(2):
                for ci in range(2):
                    c = 2 * g + ci
                    nc.tensor.matmul(
                        ps[0:128, half * QB:(half + 1) * QB],
                        lhsT=bd[:, c, 0:128],
                        rhs=qT[c][:, half * QB:(half + 1) * QB],
                        start=(ci == 0), stop=(ci == 1))
            nc.scalar.activation(out=tabexp[g][0:97, :], in_=ps[0:97, :],
                                 func=AF.Exp, scale=SCALE)
            if PHASE_LIMIT <= 3 and g == 0:
                dbg_ps = small_p.tile([128, NTOK], F32, name="dbg_ps")
                nc.vector.tensor_copy(out=dbg_ps, in_=ps)
        # broadcast tab exp rows across partitions for the eviction fold
        # (DRAM bounce: partition_broadcast silently ignores nonzero input
        # partition offsets on hardware)
        for hp in range(4):
            for hh in range(2):
                h = 2 * hp + hh
                g, j = h // 4, h % 4
                nc.sync.dma_start(out=tdscr[h:h + 1, :],
                                  in_=tabexp[g][32 * j:32 * j + 1, :])
        for hp in range(4):
            for hh in range(2):
                h = 2 * hp + hh
                nc.sync.dma_start(
                    out=bctab[hp][hh * 64:(hh + 1) * 64, :],
                    in_=tdscr[h:h + 1, :].to_broadcast([64, NTOK]))

        if PHASE_LIMIT <= 3:
            with tc.tile_pool(name="dbg", bufs=2) as dbg_p:
                for t in range(NT):
                    fo = dbg_p.tile([128, DIM], F32, name="dbg_fo", tag="fo")
                    if t == 0:
                        nc.vector.tensor_copy(out=fo, in_=bd.rearrange("p a b -> p (a b)"))
                    elif t == 1:
                        nc.vector.tensor_copy(out=fo, in_=dbg_ps[:, 0:DIM])
                    elif t == 2:
                        nc.vector.tensor_copy(out=fo, in_=tabexp[0][:, 0:DIM])
                    else:
                        nc.vector.tensor_copy(out=fo, in_=bctab[t % 4][:, 0:DIM])
                    nc.sync.dma_start(out=out_d[t * 128:(t + 1) * 128, :], in_=fo)
            return

        # ---------------- phase 4+5: attention, normalize, final (qb-outer) ----
        ae_p = ctx.enter_context(tc.tile_pool(name="aep", bufs=2))
        bc_p = ctx.enter_context(tc.tile_pool(name="bcp", bufs=2))
        fo_p = ctx.enter_context(tc.tile_pool(name="fout", bufs=2))

        for qb in range(NQB):
            for hp in range(4):
                ae0 = ae_p.tile([128, 8 * QB], BF16, name="ae0", tag="ae0")
                ae1 = ae_p.tile([128, 8 * QB], BF16, name="ae1", tag="ae1")
                qs_ = slice(qb * QB, (qb + 1) * QB)
                po = [psum_o.tile([65, QB], F32, name=f"po{hh}", tag=f"o{hh}")
                      for hh in range(2)]

                def dots_pair(kp):
                    ps0 = big_tile("psd0")
                    ps1 = big_tile("psd1")
                    for i, kt in enumerate((2 * kp, 2 * kp + 1)):
                        for hh, ps in ((0, ps0), (1, ps1)):
                            hb = hh * 64
                            nc.tensor.matmul(
                                ps[:, i * QB:(i + 1) * QB],
                                lhsT=kT[hp][hb:hb + 64, kt * 128:(kt + 1) * 128],
                                rhs=qT[hp][hb:hb + 64, qb * QB:(qb + 1) * QB],
                                start=True, stop=True)
                    nc.scalar.activation(
                        out=ae0[:, 2 * kp * QB:(2 * kp + 2) * QB],
                        in_=ps0, func=AF.Exp, scale=SCALE)
                    nc.scalar.activation(
                        out=ae1[:, 2 * kp * QB:(2 * kp + 2) * QB],
                        in_=ps1, func=AF.Exp, scale=SCALE)

                def av_pair(kp):
                    # attn@v for kpos chunk pair kp (exp'd one pair earlier)
                    for hh, ae in ((0, ae0), (1, ae1)):
                        h = 2 * hp + hh
                        for kt in (2 * kp, 2 * kp + 1):
                            nc.tensor.matmul(
                                po[hh],
                                lhsT=v_sb[kt][:, 65 * h:65 * h + 65],
                                rhs=ae[:, kt * QB:(kt + 1) * QB],
                                start=(kt == 0), stop=(kt == NT - 1))

                # interleave dots (ACT-coupled) with attn@v (no ACT) so the
                # PE never outruns the exp stream by more than one pair
                dots_pair(0)
                dots_pair(1)
                av_pair(0)
                dots_pair(2)
                av_pair(1)
                dots_pair(3)
                av_pair(2)
                av_pair(3)

                for hh in range(2):
                    h = 2 * hp + hh
                    # denominator row first (= tabexp + po[64]) so the
                    # reciprocal can fire before the outuT evictions
                    g, j = h // 4, h % 4
                    nc.vector.scalar_tensor_tensor(
                        out=dcol[g][32 * j:32 * j + 1, qs_],
                        in0=tabexp[g][32 * j:32 * j + 1, qs_],
                        scalar=1.0,
                        in1=po[hh][64:65, :],
                        op0=OP.mult, op1=OP.add)
                    # eviction with fused tab term:
                    # outuT rows = bctab*v_t_scalar + po[0:64]
                    nc.vector.scalar_tensor_tensor(
                        out=outuT[hp][hh * 64:(hh + 1) * 64, qs_],
                        in0=bctab[hp][hh * 64:(hh + 1) * 64, qs_],
                        scalar=kv_tT[hh * 64:(hh + 1) * 64, 4 + hp:5 + hp],
                        in1=po[hh][0:64, :],
                        op0=OP.mult, op1=OP.add)

                # after hp 1/3, head-group g's denominators are complete:
                # recip -> partition broadcast -> normalize
                if hp in (1, 3):
                    g = hp // 2
                    nc.vector.reciprocal_approx_accurate(
                        out=rcol[g][0:97, qs_], in_=dcol[g][0:97, qs_],
                        scratch=rscratch[0:97, 0:QB])
                    last = (qb == NQB - 1 and hp == 3)
                    for j in range(4):
                        h = 4 * g + j
                        beng = nc.scalar if (last and j % 2 == 1) else nc.sync
                        beng.dma_start(out=rdscr[h:h + 1, qs_],
                                       in_=rcol[g][32 * j:32 * j + 1, qs_])
                    for pp in range(2):
                        hp_ = 2 * g + pp
                        h0_, h1_ = 2 * hp_, 2 * hp_ + 1
                        bc = bc_p.tile([128, QB], F32, name="bc", tag=f"bc{pp}")
                        beng = nc.scalar if (last and pp == 1) else nc.sync
                        beng.dma_start(
                            out=bc[0:64, :],
                            in_=rdscr[h0_:h0_ + 1, qs_].to_broadcast([64, QB]))
                        beng.dma_start(
                            out=bc[64:128, :],
                            in_=rdscr[h1_:h1_ + 1, qs_].to_broadcast([64, QB]))
                        nc.vector.tensor_tensor(
                            out=outuT[hp_][:, qs_],
                            in0=outuT[hp_][:, qs_],
                            in1=bc, op=OP.mult)

            # ---- final projection for this q-block ----
            for t in range(4 * qb, 4 * qb + 4):
                pf = psum_f.tile([128, DIM], F32, name="pf", tag="fin")
                for c in range(NC_):
                    nc.tensor.matmul(
                        pf,
                        lhsT=outuT[c][:, t * 128:(t + 1) * 128],
                        rhs=wo[c],
                        start=(c == 0), stop=(c == NC_ - 1))
                fo = fo_p.tile([128, DIM], F32, name="fo", tag="fo")
                nc.vector.tensor_tensor(out=fo, in0=pf, in1=bout_bc, op=OP.add)
                nc.sync.dma_start(out=out_d[t * 128:(t + 1) * 128, :], in_=fo)


_CACHED_NC = None


def kernel(**inputs):
    global _CACHED_NC
    img = np.ascontiguousarray(np.asarray(inputs["img"], dtype=np.float32))
    tab = np.ascontiguousarray(np.asarray(inputs["tab"], dtype=np.float32))
    w_qkv = np.ascontiguousarray(np.asarray(inputs["w_qkv"], dtype=np.float32))
    w_tab_qkv = np.asarray(inputs["w_tab_qkv"], dtype=np.float32)
    w_out = np.ascontiguousarray(np.asarray(inputs["w_out"], dtype=np.float32))
    b_out = np.asarray(inputs["b_out"], dtype=np.float32).reshape(1, DIM)
    ln_w = np.asarray(inputs["ln_w"], dtype=np.float32).reshape(1, DIM)
    ln_b = np.asarray(inputs["ln_b"], dtype=np.float32).reshape(1, DIM)
    w_tab = np.ascontiguousarray(w_tab_qkv[:, INNER:3 * INNER])

    if _CACHED_NC is None:
        _CACHED_NC = build_program()
    nc = _CACHED_NC

    in_maps = []
    for b in range(N_CORES):
        in_maps.append({
            "img_s": np.ascontiguousarray(img[b]),
            "tab_s": np.ascontiguousarray(tab[b]),
            "w_qkv": w_qkv,
            "w_tab": w_tab,
            "w_out": w_out,
            "b_out": b_out,
            "ln_w": ln_w,
            "ln_b": ln_b,
        })

    res = bass_utils.run_bass_kernel_spmd(nc, in_maps, core_ids=list(range(N_CORES)))
    out = np.stack([res.results[c]["out_s"] for c in range(N_CORES)], axis=0)
    return out.astype(np.float32)


if __name__ == "__main__":
    d = np.load("/root/problem/ref_data.npz")
    ins = {k: d[k] for k in ("img", "tab", "w_qkv", "w_tab_qkv", "w_out",
                             "b_out", "ln_w", "ln_b")}
    actual = kernel(**ins)
    expected = d["expected"]
    err = np.abs(actual - expected).max()
    rel = err / np.abs(expected).max()
    print("absmax err:", err, "rel:", rel)
